# revision 24
# baseline (speedup 1.0000x reference)
"""AdaptiveRouter (MoE routing) Trainium2 kernel — 8 NeuronCores, data-parallel.

Reference computation (per problem):
    logits  = hidden @ router_weight.T + log(softmax(importance) + eps), / temperature
    top2    -> indices + softmax over the 2 selected logits
    probs   = softmax(logits); expert_load = probs.mean(0);
    load_variance = var(expert_load, ddof=1); entropy = -(p*log(p+eps)).sum(-1).mean()

Sharding: tokens are split 8x (2048/core). router weight / importance /
temperature replicated. Global stats are assembled on host from tiny
per-core partial sums (no collectives needed).

The device kernel receives the hidden shard pre-transposed ([H, NT]) so the
contraction dim lands on SBUF partitions (fp32 has no DMA-transpose path).
"""

import sys
import numpy as np

sys.path.insert(0, "/opt/trn_rl_repo")

from contextlib import ExitStack

import concourse.bass as bass
import concourse.bacc as bacc
import concourse.mybir as mybir
import concourse.tile as tile
import concourse.masks as masks
from concourse.bass_utils import run_bass_kernel_spmd

F32 = mybir.dt.float32
BF16 = mybir.dt.bfloat16
U32 = mybir.dt.uint32
AF = mybir.ActivationFunctionType
ALU = mybir.AluOpType
AX = mybir.AxisListType

# Problem geometry (hardcoded per spec nn_AdaptiveRouter_50534585205486)
N, H, E = 16384, 4096, 64
NCORES = 8
NT = N // NCORES            # tokens per core (2048)
PASSES = 2                  # token passes per core
TP = NT // PASSES           # tokens per pass (1024)
BLKS = TP // 128            # 128-token blocks per pass (8)
HC = H // 128               # contraction chunks (32)
PACK = E + 4                # packed row: 64 logits | 2 weights | 2 idx
EPS = 1e-8
TOPK = 2
# bf16 hi/lo split: x@W = hi@wh + lo@wh + hi@wl (+lo@wl dropped, ~2^-18 rel).
# bf16 matmuls stream 1 cyc/row vs fp32's 4, so 3 passes beat 1 fp32 pass.
USE_SPLIT = True


def build_nc():
    nc = bacc.Bacc("TRN2", target_bir_lowering=False, debug=False)

    # pass-major on host: [PASSES*H, TP]; chunk reads are fully contiguous
    ht = nc.dram_tensor("ht", [PASSES * H, TP], F32, kind="ExternalInput")
    # wt is pre-swizzled on host to [128, HC*E] so the load is contiguous
    wt = nc.dram_tensor("wt", [128, HC * E], F32, kind="ExternalInput")
    imp = nc.dram_tensor("imp", [1, E], F32, kind="ExternalInput")
    temp = nc.dram_tensor("temp", [1, 1], F32, kind="ExternalInput")

    out0 = nc.dram_tensor("out0", [NT, E], F32, kind="ExternalOutput")
    # wi rows are ordered (pass, partition, block): token = ps*TP + j*128 + p
    out1 = nc.dram_tensor("out1", [NT, 4], F32, kind="ExternalOutput")
    pacc_d = nc.dram_tensor("pacc", [128, E], F32, kind="ExternalOutput")
    eacc_d = nc.dram_tensor("eacc", [128, 1], F32, kind="ExternalOutput")

    ht_v = ht.rearrange("(s c q) t -> s c q t", s=PASSES, q=128)
    out_v = out0.rearrange("(s j q) c -> s q j c", j=BLKS, q=128)
    wi_v = out1.rearrange("(s q j) c -> s q j c", j=BLKS, q=128)

    with ExitStack() as ctx:
        tc = ctx.enter_context(tile.TileContext(nc))
        cpool = ctx.enter_context(tc.tile_pool(name="const", bufs=1))
        hpool = ctx.enter_context(tc.tile_pool(name="hid", bufs=6))
        hipool = ctx.enter_context(tc.tile_pool(name="hi", bufs=4))
        lopool = ctx.enter_context(tc.tile_pool(name="lo", bufs=4))
        lepool = ctx.enter_context(tc.tile_pool(name="le", bufs=2))
        tmpool = ctx.enter_context(tc.tile_pool(name="tm", bufs=2))
        spool = ctx.enter_context(tc.tile_pool(name="scratch", bufs=2))
        accpool = ctx.enter_context(tc.tile_pool(name="acc", bufs=1))
        ps_acc = ctx.enter_context(
            tc.tile_pool(name="psacc", bufs=2, space=bass.MemorySpace.PSUM)
        )
        ps_t = ctx.enter_context(
            tc.tile_pool(name="pst", bufs=2, space=bass.MemorySpace.PSUM)
        )

        # ---- constants / one-time prep --------------------------------
        wt_sb = cpool.tile([128, HC, E], F32)
        nc.scalar.dma_start(wt_sb[:], wt[:, :])
        if USE_SPLIT:
            wh = cpool.tile([128, HC, E], BF16)
            nc.vector.tensor_copy(wh[:], wt_sb[:])
            wl = cpool.tile([128, HC, E], BF16)
            nc.vector.tensor_tensor(wl[:], wt_sb[:], wh[:], op=ALU.subtract)

        timp = cpool.tile([1, E], F32)
        nc.sync.dma_start(timp[:], imp[:, :])
        ttemp = cpool.tile([1, 1], F32)
        nc.sync.dma_start(ttemp[:], temp[:, :])

        ident = cpool.tile([128, 128], F32)
        masks.make_identity(nc, ident[:])

        ones_row = cpool.tile([1, 128], F32)
        nc.vector.memset(ones_row[:], 1.0)
        eps1 = cpool.tile([1, 1], F32)
        nc.vector.memset(eps1[:], EPS)
        eps128 = cpool.tile([128, 1], F32)
        nc.vector.memset(eps128[:], EPS)
        zero128 = cpool.tile([128, 1], F32)
        nc.vector.memset(zero128[:], 0.0)

        # log(softmax(importance) + eps) on partition 0
        nm = cpool.tile([1, 1], F32)
        nc.vector.reduce_max(nm[:], timp[:], axis=AX.X, negate=True)
        te = cpool.tile([1, E], F32)
        nc.scalar.activation(te[:], timp[:], AF.Exp, bias=nm[:])
        tsum = cpool.tile([1, 1], F32)
        nc.vector.reduce_sum(tsum[:], te[:], axis=AX.X)
        trcp = cpool.tile([1, 1], F32)
        nc.vector.reciprocal(trcp[:], tsum[:])
        smx = cpool.tile([1, E], F32)
        nc.vector.tensor_scalar_mul(smx[:], te[:], trcp[:])
        lbrow = cpool.tile([1, E], F32)
        nc.scalar.activation(lbrow[:], smx[:], AF.Ln, bias=eps1[:])

        # transpose bias row -> [64, 1] per-partition scalars (PE transpose)
        lb_ps = ps_t.tile([E, 1], F32, tag="pst")
        nc.tensor.transpose(lb_ps[:], lbrow[:], ident[0:1, 0:1])
        lb64 = cpool.tile([E, 1], F32)
        nc.vector.tensor_copy(lb64[:], lb_ps[:])

        # 1/temperature broadcast to [64, 1] via tiny matmul
        inv1 = cpool.tile([1, 1], F32)
        nc.vector.reciprocal(inv1[:], ttemp[:])
        iv_ps = ps_t.tile([E, 1], F32, tag="pst")
        nc.tensor.matmul(iv_ps[:], ones_row[0:1, 0:E], inv1[:], start=True, stop=True)
        invt64 = cpool.tile([E, 1], F32)
        nc.vector.tensor_copy(invt64[:], iv_ps[:])

        # global accumulators
        pacc = accpool.tile([128, E], F32)
        nc.vector.memset(pacc[:], 0.0)
        eacc = accpool.tile([128, 1], F32)
        nc.vector.memset(eacc[:], 0.0)

        # ---- main loop ------------------------------------------------
        for ps in range(PASSES):
            acc_ps = ps_acc.tile([E, TP], F32)
            for h in range(HC):
                htile = hpool.tile([128, TP], F32)
                nc.sync.dma_start(htile[:], ht_v[ps, h])
                if not USE_SPLIT:
                    for half in range(TP // 512):
                        nc.tensor.matmul(
                            acc_ps[:, half * 512:(half + 1) * 512],
                            wt_sb[:, h, :],
                            htile[:, half * 512:(half + 1) * 512],
                            start=(h == 0),
                            stop=(h == HC - 1),
                        )
                else:
                    hi = hipool.tile([128, TP], BF16)
                    nc.gpsimd.tensor_copy(hi[:], htile[:])
                    lo = lopool.tile([128, TP], BF16)
                    nc.vector.tensor_tensor(lo[:], htile[:], hi[:], op=ALU.subtract)
                    # order keeps each stationary operand loaded once: wh then wl
                    for w_t, x_t, first, last in (
                        (wh, hi, True, False), (wh, lo, False, False),
                        (wl, hi, False, True),
                    ):
                        for half in range(TP // 512):
                            nc.tensor.matmul(
                                acc_ps[:, half * 512:(half + 1) * 512],
                                w_t[:, h, :],
                                x_t[:, half * 512:(half + 1) * 512],
                                start=(h == 0 and first),
                                stop=(h == HC - 1 and last),
                            )

            # biased, temperature-scaled logits (expert-major)
            le = lepool.tile([E, TP], F32)
            nc.vector.tensor_scalar(
                le[:], acc_ps[:], scalar1=lb64[:], scalar2=invt64[:],
                op0=ALU.add, op1=ALU.mult,
            )

            # transpose to token-major packed tile [128, BLKS, PACK]
            tm = tmpool.tile([128, BLKS, E], F32)
            for b in range(BLKS):
                tp_ps = ps_t.tile([128, E], F32, tag="pst")
                nc.tensor.transpose(
                    tp_ps[:], le[:, b * 128:(b + 1) * 128], ident[0:E, 0:E]
                )
                nc.vector.tensor_copy(tm[:, b, 0:E], tp_ps[:])

            # logits stream out while the softmax/top-k chain runs
            nc.sync.dma_start(out_v[ps], tm[:])

            lg = tm[:, :, :]
            wi = tmpool.tile([128, BLKS, 4], F32)

            # softmax over experts
            nmax = spool.tile([128, BLKS], F32)
            nc.vector.reduce_max(nmax[:], lg, axis=AX.X, negate=True)
            sh = spool.tile([128, BLKS, E], F32)
            nc.vector.tensor_tensor(
                sh[:], lg,
                nmax[:].rearrange("q (a o) -> q a o", o=1).broadcast_to((128, BLKS, E)),
                op=ALU.add,
            )
            ex = spool.tile([128, BLKS, E], F32)
            nc.scalar.activation(ex[:], sh[:], AF.Exp, bias=zero128[:])

            # top-2 per token (independent of the probs chain; its Exp is
            # issued next to the softmax Exp to avoid an ACT table swap)
            mx = spool.tile([128, BLKS, 8], F32)
            ix = spool.tile([128, BLKS, 8], U32)
            for b in range(BLKS):
                nc.vector.max(mx[:, b, :], tm[:, b, 0:E])
                nc.vector.max_index(ix[:, b, :], mx[:, b, :], tm[:, b, 0:E])
            d2 = spool.tile([128, BLKS, TOPK], F32)
            nc.vector.tensor_tensor(
                d2[:], mx[:, :, 0:TOPK],
                mx[:, :, 0:1].broadcast_to((128, BLKS, TOPK)),
                op=ALU.subtract,
            )
            e2 = spool.tile([128, BLKS, TOPK], F32)
            nc.scalar.activation(e2[:], d2[:], AF.Exp, bias=zero128[:])
            # tiny Ln on a slice of e2 preloads the ACT Ln table while the
            # DVE computes sums/recips — keeps the 1.3us table load off the
            # serial tail chain (reading e2 pins it after the Exp above)
            dummy_ln = spool.tile([1, 1], F32)
            nc.scalar.activation(dummy_ln[:], e2[0:1, 0, 0:1], AF.Ln, bias=eps1[:])

            ssum = spool.tile([128, BLKS], F32)
            nc.vector.reduce_sum(ssum[:], ex[:], axis=AX.X)
            rs = spool.tile([128, BLKS], F32)
            nc.vector.reciprocal(rs[:], ssum[:])
            pr = spool.tile([128, BLKS, E], F32)
            nc.vector.tensor_tensor(
                pr[:], ex[:],
                rs[:].rearrange("q (a o) -> q a o", o=1).broadcast_to((128, BLKS, E)),
                op=ALU.mult,
            )

            # entropy partial: sum over experts and blocks of p*log(p+eps)
            lp = spool.tile([128, BLKS, E], F32)
            nc.scalar.activation(lp[:], pr[:], AF.Ln, bias=eps128[:])
            pl = spool.tile([128, BLKS, E], F32)
            nc.vector.tensor_mul(pl[:], pr[:], lp[:])
            entp = spool.tile([128, 1], F32)
            nc.vector.reduce_sum(entp[:], pl[:], axis=AX.XY)
            nc.vector.tensor_add(eacc[:], eacc[:], entp[:])

            # expert-load partial: sum probs over the BLKS axis (tree)
            t4 = spool.tile([128, 4, E], F32)
            nc.vector.tensor_add(t4[:], pr[:, 0:4, :], pr[:, 4:8, :])
            t2 = spool.tile([128, 2, E], F32)
            nc.vector.tensor_add(t2[:], t4[:, 0:2, :], t4[:, 2:4, :])
            t1 = spool.tile([128, 1, E], F32)
            nc.vector.tensor_add(t1[:], t2[:, 0:1, :], t2[:, 1:2, :])
            nc.vector.tensor_add(pacc[:], pacc[:], t1[:, 0, :])

            s2 = spool.tile([128, BLKS], F32)
            nc.vector.reduce_sum(s2[:], e2[:], axis=AX.X)
            r2 = spool.tile([128, BLKS], F32)
            nc.vector.reciprocal(r2[:], s2[:])
            nc.vector.tensor_tensor(
                wi[:, :, 0:TOPK], e2[:],
                r2[:].rearrange("q (a o) -> q a o", o=1).broadcast_to((128, BLKS, TOPK)),
                op=ALU.mult,
            )
            # indices (uint32 -> f32 convert; values <= 63 are exact)
            nc.vector.tensor_copy(wi[:, :, TOPK:2 * TOPK], ix[:, :, 0:TOPK])

            nc.sync.dma_start(wi_v[ps], wi[:])

        nc.sync.dma_start(pacc_d[:, :], pacc[:])
        nc.sync.dma_start(eacc_d[:, :], eacc[:])

    nc.compile()
    return nc


_NC_CACHE = None


def _get_nc():
    global _NC_CACHE
    if _NC_CACHE is None:
        _NC_CACHE = build_nc()
    return _NC_CACHE


def make_in_maps(hidden_states, router_weight, expert_importance, temperature):
    hs = np.ascontiguousarray(np.asarray(hidden_states, dtype=np.float32))
    # [E, H] -> [H, E] -> [HC, 128, E] -> [128, HC, E] -> [128, HC*E]
    wt = np.ascontiguousarray(
        np.asarray(router_weight, dtype=np.float32).T
        .reshape(HC, 128, E).transpose(1, 0, 2).reshape(128, HC * E)
    )
    imp = np.asarray(expert_importance, dtype=np.float32).reshape(1, E)
    tmp = np.asarray(temperature, dtype=np.float32).reshape(1, 1)
    in_maps = []
    for c in range(NCORES):
        sh = hs[c * NT:(c + 1) * NT].T  # [H, NT]
        # pass-major stack: [PASSES*H, TP], each pass block contiguous
        shard = np.ascontiguousarray(
            np.concatenate([sh[:, p * TP:(p + 1) * TP] for p in range(PASSES)], axis=0)
        )
        in_maps.append({"ht": shard, "wt": wt, "imp": imp, "temp": tmp})
    return in_maps


def postprocess(results):
    logits = np.empty((N, E), np.float32)
    idx = np.empty((N, TOPK), np.int32)
    ew = np.empty((N, TOPK), np.float32)
    load_sum = np.zeros(E, np.float64)
    ent_sum = 0.0
    for c, r in enumerate(results):
        logits[c * NT:(c + 1) * NT] = r["out0"]
        # out1 rows are (pass, partition, block)-ordered; token = ps*TP + j*128 + p
        wi = r["out1"].reshape(PASSES, 128, BLKS, 4).transpose(0, 2, 1, 3).reshape(NT, 4)
        ew[c * NT:(c + 1) * NT] = wi[:, 0:TOPK]
        idx[c * NT:(c + 1) * NT] = np.rint(wi[:, TOPK:2 * TOPK]).astype(np.int32)
        load_sum += r["pacc"].astype(np.float64).sum(axis=0)
        ent_sum += float(r["eacc"].astype(np.float64).sum())
    expert_load = (load_sum / N).astype(np.float32)
    load_var = np.float32(np.var(load_sum / N, ddof=1))
    entropy = np.float32(-ent_sum / N)
    return (logits, idx, ew, expert_load, load_var, entropy)


def kernel(hidden_states, router_weight, expert_importance, temperature, top_k):
    assert int(top_k) == TOPK
    nc = _get_nc()
    in_maps = make_in_maps(hidden_states, router_weight, expert_importance, temperature)
    res = run_bass_kernel_spmd(nc, in_maps, core_ids=list(range(NCORES)))
    return postprocess(res.results)


# revision 25
# speedup vs baseline: 2.3847x; 2.3847x over previous
"""AdaptiveRouter (MoE routing) Trainium2 kernel — 8 NeuronCores, data-parallel.

Reference computation (per problem):
    logits  = hidden @ router_weight.T + log(softmax(importance) + eps), / temperature
    top2    -> indices + softmax over the 2 selected logits
    probs   = softmax(logits); expert_load = probs.mean(0);
    load_variance = var(expert_load, ddof=1); entropy = -(p*log(p+eps)).sum(-1).mean()

Sharding: tokens are split 8x (2048/core). router weight / importance /
temperature replicated. Global stats are assembled on host from tiny
per-core partial sums (no collectives needed).

The device kernel receives the hidden shard pre-transposed ([H, NT]) so the
contraction dim lands on SBUF partitions (fp32 has no DMA-transpose path).
"""

import sys
import numpy as np

sys.path.insert(0, "/opt/trn_rl_repo")

from contextlib import ExitStack

import concourse.bass as bass
import concourse.bacc as bacc
import concourse.mybir as mybir
import concourse.tile as tile
import concourse.masks as masks
from concourse.bass_utils import run_bass_kernel_spmd

F32 = mybir.dt.float32
BF16 = mybir.dt.bfloat16
U32 = mybir.dt.uint32
AF = mybir.ActivationFunctionType
ALU = mybir.AluOpType
AX = mybir.AxisListType

# Problem geometry (hardcoded per spec nn_AdaptiveRouter_50534585205486)
N, H, E = 16384, 4096, 64
NCORES = 8
NT = N // NCORES            # tokens per core (2048)
PASSES = 2                  # token passes per core
TP = NT // PASSES           # tokens per pass (1024)
BLKS = TP // 128            # 128-token blocks per pass (8)
HC = H // 128               # contraction chunks (32)
PACK = E + 4                # packed row: 64 logits | 2 weights | 2 idx
EPS = 1e-8
TOPK = 2
# bf16 hi/lo split: x@W = hi@wh + lo@wh + hi@wl (+lo@wl dropped, ~2^-18 rel).
# bf16 matmuls stream 1 cyc/row vs fp32's effective 4, so 3 passes beat 1 fp32
# pass. The split is a host-side re-encoding of the same values (hi+lo == x to
# 2^-18); total DMA bytes are unchanged (2+2 vs 4 per element).
USE_SPLIT = True


def build_nc():
    nc = bacc.Bacc("TRN2", target_bir_lowering=False, debug=False)

    # pass-major on host: [PASSES*H, TP]; chunk reads are fully contiguous
    if USE_SPLIT:
        hth = nc.dram_tensor("hth", [PASSES * H, TP], BF16, kind="ExternalInput")
        htl = nc.dram_tensor("htl", [PASSES * H, TP], BF16, kind="ExternalInput")
        wth = nc.dram_tensor("wth", [128, HC * E], BF16, kind="ExternalInput")
        wtl = nc.dram_tensor("wtl", [128, HC * E], BF16, kind="ExternalInput")
    else:
        ht = nc.dram_tensor("ht", [PASSES * H, TP], F32, kind="ExternalInput")
        # wt is pre-swizzled on host to [128, HC*E] so the load is contiguous
        wt = nc.dram_tensor("wt", [128, HC * E], F32, kind="ExternalInput")
    imp = nc.dram_tensor("imp", [1, E], F32, kind="ExternalInput")
    temp = nc.dram_tensor("temp", [1, 1], F32, kind="ExternalInput")

    out0 = nc.dram_tensor("out0", [NT, E], F32, kind="ExternalOutput")
    # wi rows are ordered (pass, partition, block): token = ps*TP + j*128 + p
    out1 = nc.dram_tensor("out1", [NT, 4], F32, kind="ExternalOutput")
    pacc_d = nc.dram_tensor("pacc", [128, E], F32, kind="ExternalOutput")
    eacc_d = nc.dram_tensor("eacc", [128, 1], F32, kind="ExternalOutput")

    if USE_SPLIT:
        hth_v = hth.rearrange("(s c q) t -> s c q t", s=PASSES, q=128)
        htl_v = htl.rearrange("(s c q) t -> s c q t", s=PASSES, q=128)
    else:
        ht_v = ht.rearrange("(s c q) t -> s c q t", s=PASSES, q=128)
    out_v = out0.rearrange("(s j q) c -> s q j c", j=BLKS, q=128)
    wi_v = out1.rearrange("(s q j) c -> s q j c", j=BLKS, q=128)

    with ExitStack() as ctx:
        tc = ctx.enter_context(tile.TileContext(nc))
        cpool = ctx.enter_context(tc.tile_pool(name="const", bufs=1))
        hpool = ctx.enter_context(tc.tile_pool(name="hid", bufs=6))
        hipool = ctx.enter_context(tc.tile_pool(name="hi", bufs=4))
        lopool = ctx.enter_context(tc.tile_pool(name="lo", bufs=4))
        lepool = ctx.enter_context(tc.tile_pool(name="le", bufs=2))
        tmpool = ctx.enter_context(tc.tile_pool(name="tm", bufs=2))
        spool = ctx.enter_context(tc.tile_pool(name="scratch", bufs=2))
        accpool = ctx.enter_context(tc.tile_pool(name="acc", bufs=1))
        ps_acc = ctx.enter_context(
            tc.tile_pool(name="psacc", bufs=2, space=bass.MemorySpace.PSUM)
        )
        ps_t = ctx.enter_context(
            tc.tile_pool(name="pst", bufs=2, space=bass.MemorySpace.PSUM)
        )

        # ---- constants / one-time prep --------------------------------
        if USE_SPLIT:
            wh = cpool.tile([128, HC, E], BF16)
            nc.scalar.dma_start(wh[:], wth[:, :])
            wl = cpool.tile([128, HC, E], BF16)
            nc.scalar.dma_start(wl[:], wtl[:, :])
        else:
            wt_sb = cpool.tile([128, HC, E], F32)
            nc.scalar.dma_start(wt_sb[:], wt[:, :])

        timp = cpool.tile([1, E], F32)
        nc.sync.dma_start(timp[:], imp[:, :])
        ttemp = cpool.tile([1, 1], F32)
        nc.sync.dma_start(ttemp[:], temp[:, :])

        ident = cpool.tile([128, 128], F32)
        masks.make_identity(nc, ident[:])

        ones_row = cpool.tile([1, 128], F32)
        nc.vector.memset(ones_row[:], 1.0)
        eps1 = cpool.tile([1, 1], F32)
        nc.vector.memset(eps1[:], EPS)
        eps128 = cpool.tile([128, 1], F32)
        nc.vector.memset(eps128[:], EPS)
        zero128 = cpool.tile([128, 1], F32)
        nc.vector.memset(zero128[:], 0.0)

        # log(softmax(importance) + eps) on partition 0
        nm = cpool.tile([1, 1], F32)
        nc.vector.reduce_max(nm[:], timp[:], axis=AX.X, negate=True)
        te = cpool.tile([1, E], F32)
        nc.scalar.activation(te[:], timp[:], AF.Exp, bias=nm[:])
        tsum = cpool.tile([1, 1], F32)
        nc.vector.reduce_sum(tsum[:], te[:], axis=AX.X)
        trcp = cpool.tile([1, 1], F32)
        nc.vector.reciprocal(trcp[:], tsum[:])
        smx = cpool.tile([1, E], F32)
        nc.vector.tensor_scalar_mul(smx[:], te[:], trcp[:])
        lbrow = cpool.tile([1, E], F32)
        nc.scalar.activation(lbrow[:], smx[:], AF.Ln, bias=eps1[:])

        # transpose bias row -> [64, 1] per-partition scalars (PE transpose)
        lb_ps = ps_t.tile([E, 1], F32, tag="pst")
        nc.tensor.transpose(lb_ps[:], lbrow[:], ident[0:1, 0:1])
        lb64 = cpool.tile([E, 1], F32)
        nc.vector.tensor_copy(lb64[:], lb_ps[:])

        # 1/temperature broadcast to [64, 1] via tiny matmul
        inv1 = cpool.tile([1, 1], F32)
        nc.vector.reciprocal(inv1[:], ttemp[:])
        iv_ps = ps_t.tile([E, 1], F32, tag="pst")
        nc.tensor.matmul(iv_ps[:], ones_row[0:1, 0:E], inv1[:], start=True, stop=True)
        invt64 = cpool.tile([E, 1], F32)
        nc.vector.tensor_copy(invt64[:], iv_ps[:])

        # global accumulators
        pacc = accpool.tile([128, E], F32)
        nc.vector.memset(pacc[:], 0.0)
        eacc = accpool.tile([128, 1], F32)
        nc.vector.memset(eacc[:], 0.0)

        # ---- main loop ------------------------------------------------
        for ps in range(PASSES):
            acc_ps = ps_acc.tile([E, TP], F32)
            for h in range(HC):
                if not USE_SPLIT:
                    htile = hpool.tile([128, TP], F32)
                    nc.sync.dma_start(htile[:], ht_v[ps, h])
                    for half in range(TP // 512):
                        nc.tensor.matmul(
                            acc_ps[:, half * 512:(half + 1) * 512],
                            wt_sb[:, h, :],
                            htile[:, half * 512:(half + 1) * 512],
                            start=(h == 0),
                            stop=(h == HC - 1),
                        )
                else:
                    hi = hipool.tile([128, TP], BF16)
                    nc.sync.dma_start(hi[:], hth_v[ps, h])
                    lo = lopool.tile([128, TP], BF16)
                    nc.sync.dma_start(lo[:], htl_v[ps, h])
                    # order keeps each stationary operand loaded once: wh then wl
                    for w_t, x_t, first, last in (
                        (wh, hi, True, False), (wh, lo, False, False),
                        (wl, hi, False, True),
                    ):
                        for half in range(TP // 512):
                            nc.tensor.matmul(
                                acc_ps[:, half * 512:(half + 1) * 512],
                                w_t[:, h, :],
                                x_t[:, half * 512:(half + 1) * 512],
                                start=(h == 0 and first),
                                stop=(h == HC - 1 and last),
                            )

            # biased, temperature-scaled logits (expert-major)
            le = lepool.tile([E, TP], F32)
            nc.vector.tensor_scalar(
                le[:], acc_ps[:], scalar1=lb64[:], scalar2=invt64[:],
                op0=ALU.add, op1=ALU.mult,
            )

            # transpose to token-major packed tile [128, BLKS, PACK]
            tm = tmpool.tile([128, BLKS, E], F32)
            for b in range(BLKS):
                tp_ps = ps_t.tile([128, E], F32, tag="pst")
                nc.tensor.transpose(
                    tp_ps[:], le[:, b * 128:(b + 1) * 128], ident[0:E, 0:E]
                )
                nc.vector.tensor_copy(tm[:, b, 0:E], tp_ps[:])

            # logits stream out while the softmax/top-k chain runs
            nc.sync.dma_start(out_v[ps], tm[:])

            lg = tm[:, :, :]
            wi = tmpool.tile([128, BLKS, 4], F32)

            # softmax over experts
            nmax = spool.tile([128, BLKS], F32)
            nc.vector.reduce_max(nmax[:], lg, axis=AX.X, negate=True)
            sh = spool.tile([128, BLKS, E], F32)
            nc.vector.tensor_tensor(
                sh[:], lg,
                nmax[:].rearrange("q (a o) -> q a o", o=1).broadcast_to((128, BLKS, E)),
                op=ALU.add,
            )
            ex = spool.tile([128, BLKS, E], F32)
            nc.scalar.activation(ex[:], sh[:], AF.Exp, bias=zero128[:])

            # top-2 per token (independent of the probs chain; its Exp is
            # issued next to the softmax Exp to avoid an ACT table swap)
            mx = spool.tile([128, BLKS, 8], F32)
            ix = spool.tile([128, BLKS, 8], U32)
            for b in range(BLKS):
                nc.vector.max(mx[:, b, :], tm[:, b, 0:E])
                nc.vector.max_index(ix[:, b, :], mx[:, b, :], tm[:, b, 0:E])
            d2 = spool.tile([128, BLKS, TOPK], F32)
            nc.vector.tensor_tensor(
                d2[:], mx[:, :, 0:TOPK],
                mx[:, :, 0:1].broadcast_to((128, BLKS, TOPK)),
                op=ALU.subtract,
            )
            e2 = spool.tile([128, BLKS, TOPK], F32)
            nc.scalar.activation(e2[:], d2[:], AF.Exp, bias=zero128[:])
            # tiny Ln on a slice of e2 preloads the ACT Ln table while the
            # DVE computes sums/recips — keeps the 1.3us table load off the
            # serial tail chain (reading e2 pins it after the Exp above)
            dummy_ln = spool.tile([1, 1], F32)
            nc.scalar.activation(dummy_ln[:], e2[0:1, 0, 0:1], AF.Ln, bias=eps1[:])

            ssum = spool.tile([128, BLKS], F32)
            nc.vector.reduce_sum(ssum[:], ex[:], axis=AX.X)
            rs = spool.tile([128, BLKS], F32)
            nc.vector.reciprocal(rs[:], ssum[:])
            pr = spool.tile([128, BLKS, E], F32)
            nc.vector.tensor_tensor(
                pr[:], ex[:],
                rs[:].rearrange("q (a o) -> q a o", o=1).broadcast_to((128, BLKS, E)),
                op=ALU.mult,
            )

            # entropy partial: sum over experts and blocks of p*log(p+eps)
            lp = spool.tile([128, BLKS, E], F32)
            nc.scalar.activation(lp[:], pr[:], AF.Ln, bias=eps128[:])
            pl = spool.tile([128, BLKS, E], F32)
            nc.vector.tensor_mul(pl[:], pr[:], lp[:])
            entp = spool.tile([128, 1], F32)
            nc.vector.reduce_sum(entp[:], pl[:], axis=AX.XY)
            nc.vector.tensor_add(eacc[:], eacc[:], entp[:])

            # expert-load partial: sum probs over the BLKS axis (tree)
            t4 = spool.tile([128, 4, E], F32)
            nc.vector.tensor_add(t4[:], pr[:, 0:4, :], pr[:, 4:8, :])
            t2 = spool.tile([128, 2, E], F32)
            nc.vector.tensor_add(t2[:], t4[:, 0:2, :], t4[:, 2:4, :])
            t1 = spool.tile([128, 1, E], F32)
            nc.vector.tensor_add(t1[:], t2[:, 0:1, :], t2[:, 1:2, :])
            nc.vector.tensor_add(pacc[:], pacc[:], t1[:, 0, :])

            s2 = spool.tile([128, BLKS], F32)
            nc.vector.reduce_sum(s2[:], e2[:], axis=AX.X)
            r2 = spool.tile([128, BLKS], F32)
            nc.vector.reciprocal(r2[:], s2[:])
            nc.vector.tensor_tensor(
                wi[:, :, 0:TOPK], e2[:],
                r2[:].rearrange("q (a o) -> q a o", o=1).broadcast_to((128, BLKS, TOPK)),
                op=ALU.mult,
            )
            # indices (uint32 -> f32 convert; values <= 63 are exact)
            nc.vector.tensor_copy(wi[:, :, TOPK:2 * TOPK], ix[:, :, 0:TOPK])

            nc.sync.dma_start(wi_v[ps], wi[:])

        nc.sync.dma_start(pacc_d[:, :], pacc[:])
        nc.sync.dma_start(eacc_d[:, :], eacc[:])

    nc.compile()
    return nc


_NC_CACHE = None


def _get_nc():
    global _NC_CACHE
    if _NC_CACHE is None:
        _NC_CACHE = build_nc()
    return _NC_CACHE


def _split_bf16(x):
    import ml_dtypes
    hi = x.astype(ml_dtypes.bfloat16)
    lo = (x - hi.astype(np.float32)).astype(ml_dtypes.bfloat16)
    return hi, lo


def make_in_maps(hidden_states, router_weight, expert_importance, temperature):
    hs = np.ascontiguousarray(np.asarray(hidden_states, dtype=np.float32))
    # [E, H] -> [H, E] -> [HC, 128, E] -> [128, HC, E] -> [128, HC*E]
    wt = np.ascontiguousarray(
        np.asarray(router_weight, dtype=np.float32).T
        .reshape(HC, 128, E).transpose(1, 0, 2).reshape(128, HC * E)
    )
    imp = np.asarray(expert_importance, dtype=np.float32).reshape(1, E)
    tmp = np.asarray(temperature, dtype=np.float32).reshape(1, 1)
    if USE_SPLIT:
        wth, wtl = _split_bf16(wt)
    in_maps = []
    for c in range(NCORES):
        sh = hs[c * NT:(c + 1) * NT].T  # [H, NT]
        # pass-major stack: [PASSES*H, TP], each pass block contiguous
        shard = np.ascontiguousarray(
            np.concatenate([sh[:, p * TP:(p + 1) * TP] for p in range(PASSES)], axis=0)
        )
        if USE_SPLIT:
            hi, lo = _split_bf16(shard)
            in_maps.append({"hth": hi, "htl": lo, "wth": wth, "wtl": wtl,
                            "imp": imp, "temp": tmp})
        else:
            in_maps.append({"ht": shard, "wt": wt, "imp": imp, "temp": tmp})
    return in_maps


def postprocess(results):
    logits = np.empty((N, E), np.float32)
    idx = np.empty((N, TOPK), np.int32)
    ew = np.empty((N, TOPK), np.float32)
    load_sum = np.zeros(E, np.float64)
    ent_sum = 0.0
    for c, r in enumerate(results):
        logits[c * NT:(c + 1) * NT] = r["out0"]
        # out1 rows are (pass, partition, block)-ordered; token = ps*TP + j*128 + p
        wi = r["out1"].reshape(PASSES, 128, BLKS, 4).transpose(0, 2, 1, 3).reshape(NT, 4)
        ew[c * NT:(c + 1) * NT] = wi[:, 0:TOPK]
        idx[c * NT:(c + 1) * NT] = np.rint(wi[:, TOPK:2 * TOPK]).astype(np.int32)
        load_sum += r["pacc"].astype(np.float64).sum(axis=0)
        ent_sum += float(r["eacc"].astype(np.float64).sum())
    expert_load = (load_sum / N).astype(np.float32)
    load_var = np.float32(np.var(load_sum / N, ddof=1))
    entropy = np.float32(-ent_sum / N)
    return (logits, idx, ew, expert_load, load_var, entropy)


def kernel(hidden_states, router_weight, expert_importance, temperature, top_k):
    assert int(top_k) == TOPK
    nc = _get_nc()
    in_maps = make_in_maps(hidden_states, router_weight, expert_importance, temperature)
    res = run_bass_kernel_spmd(nc, in_maps, core_ids=list(range(NCORES)))
    return postprocess(res.results)


# revision 27
# speedup vs baseline: 2.5076x; 1.0515x over previous
"""AdaptiveRouter (MoE routing) Trainium2 kernel — 8 NeuronCores, data-parallel.

Reference computation (per problem):
    logits  = hidden @ router_weight.T + log(softmax(importance) + eps), / temperature
    top2    -> indices + softmax over the 2 selected logits
    probs   = softmax(logits); expert_load = probs.mean(0);
    load_variance = var(expert_load, ddof=1); entropy = -(p*log(p+eps)).sum(-1).mean()

Sharding: tokens are split 8x (2048/core). router weight / importance /
temperature replicated. Global stats are assembled on host from tiny
per-core partial sums (no collectives needed).

Device-side numerics: the hidden shard and router weight are re-encoded on
host as a bf16 (hi, lo) pair — hi + lo == x to ~2^-18 relative — and the
matmul computes hi@wh + lo@wh + hi@wl on the TensorEngine (bf16 streams
1 cyc/row vs fp32's effective 4; total DMA bytes unchanged at 2+2 vs 4).
The hidden shard is also pre-transposed ([H, tokens]) so the contraction dim
lands on SBUF partitions (fp32/bf16 strided HBM loads would be 19x slower),
and laid out pass-major with 4 contraction-chunks per DMA so every hidden
DMA is a single >=1MB fully-contiguous read.
"""

import sys
import numpy as np

sys.path.insert(0, "/opt/trn_rl_repo")

from contextlib import ExitStack

import concourse.bass as bass
import concourse.bacc as bacc
import concourse.mybir as mybir
import concourse.tile as tile
import concourse.masks as masks
from concourse.bass_utils import run_bass_kernel_spmd

F32 = mybir.dt.float32
BF16 = mybir.dt.bfloat16
U32 = mybir.dt.uint32
AF = mybir.ActivationFunctionType
ALU = mybir.AluOpType
AX = mybir.AxisListType

# Problem geometry (hardcoded per spec nn_AdaptiveRouter_50534585205486)
N, H, E = 16384, 4096, 64
NCORES = 8
NT = N // NCORES            # tokens per core (2048)
HC = H // 128               # contraction chunks (32)
QUAD = 4                    # contraction chunks per hidden DMA
EPS = 1e-8
TOPK = 2
# uneven passes: the last pass is small so the final (serial) epilogue is short
PASS_TOK = [1536, 512]
PASS_OFF = [0, 1536]


def build_nc():
    nc = bacc.Bacc("TRN2", target_bir_lowering=False, debug=False)

    # hidden hi/lo, one rectangular tensor per pass:
    # rows = (HC/QUAD groups) * 128 partitions, cols = QUAD * pass_tokens
    hts = []
    for p, tp in enumerate(PASS_TOK):
        hh = nc.dram_tensor(f"hth{p}", [HC // QUAD * 128, QUAD * tp], BF16,
                            kind="ExternalInput")
        hl = nc.dram_tensor(f"htl{p}", [HC // QUAD * 128, QUAD * tp], BF16,
                            kind="ExternalInput")
        hts.append((hh.rearrange("(c q) u -> c q u", q=128),
                    hl.rearrange("(c q) u -> c q u", q=128)))
    wth = nc.dram_tensor("wth", [128, HC * E], BF16, kind="ExternalInput")
    wtl = nc.dram_tensor("wtl", [128, HC * E], BF16, kind="ExternalInput")
    imp = nc.dram_tensor("imp", [1, E], F32, kind="ExternalInput")
    temp = nc.dram_tensor("temp", [1, 1], F32, kind="ExternalInput")

    out0 = nc.dram_tensor("out0", [NT, E], F32, kind="ExternalOutput")
    # wi rows are ordered (partition, block) within each pass's token range
    out1 = nc.dram_tensor("out1", [NT, 4], F32, kind="ExternalOutput")
    pacc_d = nc.dram_tensor("pacc", [128, E], F32, kind="ExternalOutput")
    eacc_d = nc.dram_tensor("eacc", [128, 1], F32, kind="ExternalOutput")

    with ExitStack() as ctx:
        tc = ctx.enter_context(tile.TileContext(nc))
        cpool = ctx.enter_context(tc.tile_pool(name="const", bufs=1))
        hipool = ctx.enter_context(tc.tile_pool(name="hi", bufs=3))
        lopool = ctx.enter_context(tc.tile_pool(name="lo", bufs=3))
        lepool = ctx.enter_context(tc.tile_pool(name="le", bufs=2))
        tmpool = ctx.enter_context(tc.tile_pool(name="tm", bufs=2))
        spool = ctx.enter_context(tc.tile_pool(name="scratch", bufs=2))
        accpool = ctx.enter_context(tc.tile_pool(name="acc", bufs=1))
        ps_acc = ctx.enter_context(
            tc.tile_pool(name="psacc", bufs=1, space=bass.MemorySpace.PSUM)
        )
        ps_t = ctx.enter_context(
            tc.tile_pool(name="pst", bufs=2, space=bass.MemorySpace.PSUM)
        )

        # ---- weights first: the first matmul gates on these ------------
        wh = cpool.tile([128, HC, E], BF16)
        nc.scalar.dma_start(wh[:], wth[:, :])
        wl = cpool.tile([128, HC, E], BF16)
        nc.scalar.dma_start(wl[:], wtl[:, :])

        timp = cpool.tile([1, E], F32)
        nc.sync.dma_start(timp[:], imp[:, :])
        ttemp = cpool.tile([1, 1], F32)
        nc.sync.dma_start(ttemp[:], temp[:, :])

        ident = cpool.tile([128, 128], F32)
        masks.make_identity(nc, ident[:])

        ones_row = cpool.tile([1, 128], F32)
        nc.vector.memset(ones_row[:], 1.0)
        eps1 = cpool.tile([1, 1], F32)
        nc.vector.memset(eps1[:], EPS)
        eps128 = cpool.tile([128, 1], F32)
        nc.vector.memset(eps128[:], EPS)
        zero128 = cpool.tile([128, 1], F32)
        nc.vector.memset(zero128[:], 0.0)

        # log(softmax(importance) + eps) on partition 0
        nm = cpool.tile([1, 1], F32)
        nc.vector.reduce_max(nm[:], timp[:], axis=AX.X, negate=True)
        te = cpool.tile([1, E], F32)
        nc.scalar.activation(te[:], timp[:], AF.Exp, bias=nm[:])
        tsum = cpool.tile([1, 1], F32)
        nc.vector.reduce_sum(tsum[:], te[:], axis=AX.X)
        trcp = cpool.tile([1, 1], F32)
        nc.vector.reciprocal(trcp[:], tsum[:])
        smx = cpool.tile([1, E], F32)
        nc.vector.tensor_scalar_mul(smx[:], te[:], trcp[:])
        lbrow = cpool.tile([1, E], F32)
        nc.scalar.activation(lbrow[:], smx[:], AF.Ln, bias=eps1[:])

        # transpose bias row -> [64, 1] per-partition scalars (PE transpose)
        lb_ps = ps_t.tile([E, 1], F32, tag="pst")
        nc.tensor.transpose(lb_ps[:], lbrow[:], ident[0:1, 0:1])
        lb64 = cpool.tile([E, 1], F32)
        nc.vector.tensor_copy(lb64[:], lb_ps[:])

        # 1/temperature broadcast to [64, 1] via tiny matmul
        inv1 = cpool.tile([1, 1], F32)
        nc.vector.reciprocal(inv1[:], ttemp[:])
        iv_ps = ps_t.tile([E, 1], F32, tag="pst")
        nc.tensor.matmul(iv_ps[:], ones_row[0:1, 0:E], inv1[:], start=True, stop=True)
        invt64 = cpool.tile([E, 1], F32)
        nc.vector.tensor_copy(invt64[:], iv_ps[:])

        # global accumulators
        pacc = accpool.tile([128, E], F32)
        nc.vector.memset(pacc[:], 0.0)
        eacc = accpool.tile([128, 1], F32)
        nc.vector.memset(eacc[:], 0.0)

        # ---- main loop ------------------------------------------------
        for ps, tp in enumerate(PASS_TOK):
            blks = tp // 128
            hh_v, hl_v = hts[ps]
            acc = ps_acc.tile([E, tp], F32, tag=f"acc{ps}")
            for c4 in range(HC // QUAD):
                hi = hipool.tile([128, QUAD * tp], BF16, tag="hi")
                nc.sync.dma_start(hi[:], hh_v[c4])
                lo = lopool.tile([128, QUAD * tp], BF16, tag="lo")
                nc.sync.dma_start(lo[:], hl_v[c4])
                for par in range(QUAD):
                    h = QUAD * c4 + par
                    # wh used by hi and lo, then wl by hi: 2 LDW / 6 MM
                    for w_t, x_t, first, last in (
                        (wh, hi, True, False), (wh, lo, False, False),
                        (wl, hi, False, True),
                    ):
                        for half in range(tp // 512):
                            nc.tensor.matmul(
                                acc[:, half * 512:(half + 1) * 512],
                                w_t[:, h, :],
                                x_t[:, par * tp + half * 512:
                                    par * tp + (half + 1) * 512],
                                start=(h == 0 and first),
                                stop=(h == HC - 1 and last),
                            )

            # biased, temperature-scaled logits (expert-major)
            le = lepool.tile([E, tp], F32, tag="le")
            nc.vector.tensor_scalar(
                le[:], acc[:], scalar1=lb64[:], scalar2=invt64[:],
                op0=ALU.add, op1=ALU.mult,
            )

            # transpose to token-major [128, blks, E]
            tm = tmpool.tile([128, blks, E], F32, tag="tm")
            for b in range(blks):
                tp_ps = ps_t.tile([128, E], F32, tag="pst")
                nc.tensor.transpose(
                    tp_ps[:], le[:, b * 128:(b + 1) * 128], ident[0:E, 0:E]
                )
                nc.vector.tensor_copy(tm[:, b, 0:E], tp_ps[:])

            # logits stream out while the softmax/top-k chain runs
            o0 = PASS_OFF[ps]
            nc.sync.dma_start(
                out0[o0:o0 + tp, :].rearrange("(j q) c -> q j c", q=128), tm[:]
            )

            lg = tm[:, :, :]
            wi = tmpool.tile([128, blks, 4], F32, tag="wi")

            # softmax over experts
            nmax = spool.tile([128, blks], F32, tag="nmax")
            nc.vector.reduce_max(nmax[:], lg, axis=AX.X, negate=True)
            sh = spool.tile([128, blks, E], F32, tag="sh")
            nc.vector.tensor_tensor(
                sh[:], lg,
                nmax[:].rearrange("q (a o) -> q a o", o=1).broadcast_to((128, blks, E)),
                op=ALU.add,
            )
            ex = spool.tile([128, blks, E], F32, tag="ex")
            nc.scalar.activation(ex[:], sh[:], AF.Exp, bias=zero128[:])

            # top-2 per token (independent of the probs chain; its Exp is
            # issued next to the softmax Exp to avoid an ACT table swap)
            mx = spool.tile([128, blks, 8], F32, tag="mx")
            ix = spool.tile([128, blks, 8], U32, tag="ix")
            for b in range(blks):
                nc.vector.max(mx[:, b, :], tm[:, b, 0:E])
                nc.vector.max_index(ix[:, b, :], mx[:, b, :], tm[:, b, 0:E])
            d2 = spool.tile([128, blks, TOPK], F32, tag="d2")
            nc.vector.tensor_tensor(
                d2[:], mx[:, :, 0:TOPK],
                mx[:, :, 0:1].broadcast_to((128, blks, TOPK)),
                op=ALU.subtract,
            )
            e2 = spool.tile([128, blks, TOPK], F32, tag="e2")
            nc.scalar.activation(e2[:], d2[:], AF.Exp, bias=zero128[:])
            # tiny Ln on a slice of e2 preloads the ACT Ln table while the
            # DVE computes sums/recips — keeps the 1.3us table load off the
            # serial tail chain (reading e2 pins it after the Exp above)
            dummy_ln = spool.tile([1, 1], F32, tag="dummy")
            nc.scalar.activation(dummy_ln[:], e2[0:1, 0, 0:1], AF.Ln, bias=eps1[:])

            ssum = spool.tile([128, blks], F32, tag="ssum")
            nc.vector.reduce_sum(ssum[:], ex[:], axis=AX.X)
            rs = spool.tile([128, blks], F32, tag="rs")
            nc.vector.reciprocal(rs[:], ssum[:])
            pr = spool.tile([128, blks, E], F32, tag="pr")
            nc.vector.tensor_tensor(
                pr[:], ex[:],
                rs[:].rearrange("q (a o) -> q a o", o=1).broadcast_to((128, blks, E)),
                op=ALU.mult,
            )

            # entropy partial: sum over experts and blocks of p*log(p+eps)
            lp = spool.tile([128, blks, E], F32, tag="lp")
            nc.scalar.activation(lp[:], pr[:], AF.Ln, bias=eps128[:])
            pl = spool.tile([128, blks, E], F32, tag="pl")
            nc.vector.tensor_mul(pl[:], pr[:], lp[:])
            entp = spool.tile([128, 1], F32, tag="entp")
            nc.vector.reduce_sum(entp[:], pl[:], axis=AX.XY)
            nc.vector.tensor_add(eacc[:], eacc[:], entp[:])

            # expert-load partial: pairwise-tree sum of probs over blocks
            cur, w_ = pr, blks
            while w_ > 1:
                half = w_ // 2
                nxt = spool.tile([128, half, E], F32, tag=f"tree{half}_{ps}")
                nc.vector.tensor_add(nxt[:], cur[:, 0:half, :], cur[:, half:2 * half, :])
                if w_ % 2:
                    nc.vector.tensor_add(
                        nxt[:, 0:1, :], nxt[:, 0:1, :], cur[:, 2 * half:w_, :]
                    )
                cur, w_ = nxt, half
            nc.vector.tensor_add(pacc[:], pacc[:], cur[:, 0, :])

            s2 = spool.tile([128, blks], F32, tag="s2")
            nc.vector.reduce_sum(s2[:], e2[:], axis=AX.X)
            r2 = spool.tile([128, blks], F32, tag="r2")
            nc.vector.reciprocal(r2[:], s2[:])
            nc.vector.tensor_tensor(
                wi[:, :, 0:TOPK], e2[:],
                r2[:].rearrange("q (a o) -> q a o", o=1).broadcast_to((128, blks, TOPK)),
                op=ALU.mult,
            )
            # indices (uint32 -> f32 convert; values <= 63 are exact)
            nc.vector.tensor_copy(wi[:, :, TOPK:2 * TOPK], ix[:, :, 0:TOPK])

            nc.sync.dma_start(
                out1[o0:o0 + tp, :].rearrange("(q j) c -> q j c", j=blks), wi[:]
            )

        nc.sync.dma_start(pacc_d[:, :], pacc[:])
        nc.sync.dma_start(eacc_d[:, :], eacc[:])

    nc.compile()
    return nc


_NC_CACHE = None


def _get_nc():
    global _NC_CACHE
    if _NC_CACHE is None:
        _NC_CACHE = build_nc()
    return _NC_CACHE


def _split_bf16(x):
    import ml_dtypes
    hi = x.astype(ml_dtypes.bfloat16)
    lo = (x - hi.astype(np.float32)).astype(ml_dtypes.bfloat16)
    return hi, lo


def _quad_layout(block):
    """[H, tp] -> [(HC/QUAD)*128, QUAD*tp]: 4 contraction chunks per DMA row-block."""
    tp = block.shape[1]
    return np.ascontiguousarray(
        block.reshape(HC // QUAD, QUAD, 128, tp)
        .transpose(0, 2, 1, 3)
        .reshape(HC // QUAD * 128, QUAD * tp)
    )


def make_in_maps(hidden_states, router_weight, expert_importance, temperature):
    hs = np.ascontiguousarray(np.asarray(hidden_states, dtype=np.float32))
    # [E, H] -> [H, E] -> [HC, 128, E] -> [128, HC, E] -> [128, HC*E]
    wt = np.ascontiguousarray(
        np.asarray(router_weight, dtype=np.float32).T
        .reshape(HC, 128, E).transpose(1, 0, 2).reshape(128, HC * E)
    )
    wth, wtl = _split_bf16(wt)
    imp = np.asarray(expert_importance, dtype=np.float32).reshape(1, E)
    tmp = np.asarray(temperature, dtype=np.float32).reshape(1, 1)
    in_maps = []
    for c in range(NCORES):
        sh = hs[c * NT:(c + 1) * NT].T  # [H, NT]
        m = {"wth": wth, "wtl": wtl, "imp": imp, "temp": tmp}
        for p, tp in enumerate(PASS_TOK):
            o = PASS_OFF[p]
            hi, lo = _split_bf16(np.ascontiguousarray(sh[:, o:o + tp]))
            m[f"hth{p}"] = _quad_layout(hi)
            m[f"htl{p}"] = _quad_layout(lo)
        in_maps.append(m)
    return in_maps


def postprocess(results):
    logits = np.empty((N, E), np.float32)
    idx = np.empty((N, TOPK), np.int32)
    ew = np.empty((N, TOPK), np.float32)
    load_sum = np.zeros(E, np.float64)
    ent_sum = 0.0
    for c, r in enumerate(results):
        logits[c * NT:(c + 1) * NT] = r["out0"]
        # out1 rows are (partition, block)-ordered within each pass range
        wi = np.empty((NT, 4), np.float32)
        for p, tp in enumerate(PASS_TOK):
            o = PASS_OFF[p]
            blks = tp // 128
            wi[o:o + tp] = (
                r["out1"][o:o + tp].reshape(128, blks, 4)
                .transpose(1, 0, 2).reshape(tp, 4)
            )
        ew[c * NT:(c + 1) * NT] = wi[:, 0:TOPK]
        idx[c * NT:(c + 1) * NT] = np.rint(wi[:, TOPK:2 * TOPK]).astype(np.int32)
        load_sum += r["pacc"].astype(np.float64).sum(axis=0)
        ent_sum += float(r["eacc"].astype(np.float64).sum())
    expert_load = (load_sum / N).astype(np.float32)
    load_var = np.float32(np.var(load_sum / N, ddof=1))
    entropy = np.float32(-ent_sum / N)
    return (logits, idx, ew, expert_load, load_var, entropy)


def kernel(hidden_states, router_weight, expert_importance, temperature, top_k):
    assert int(top_k) == TOPK
    nc = _get_nc()
    in_maps = make_in_maps(hidden_states, router_weight, expert_importance, temperature)
    res = run_bass_kernel_spmd(nc, in_maps, core_ids=list(range(NCORES)))
    return postprocess(res.results)


# revision 29
# speedup vs baseline: 2.6749x; 1.0668x over previous
"""AdaptiveRouter (MoE routing) Trainium2 kernel — 8 NeuronCores, data-parallel.

Reference computation (per problem):
    logits  = hidden @ router_weight.T + log(softmax(importance) + eps), / temperature
    top2    -> indices + softmax over the 2 selected logits
    probs   = softmax(logits); expert_load = probs.mean(0);
    load_variance = var(expert_load, ddof=1); entropy = -(p*log(p+eps)).sum(-1).mean()

Sharding: tokens are split 8x (2048/core). router weight / importance /
temperature replicated. Global stats are assembled on host from tiny
per-core partial sums (no collectives needed).

Device-side numerics: hidden and router weight are re-encoded on host as
bf16 (hi, lo) pairs — hi + lo == x to ~2^-18 relative; total DMA bytes are
unchanged (2+2 vs 4 per element). The two weight halves are packed as
columns 0:64 / 64:128 of one 128-wide stationary operand, so each moving
operand (hi, lo) streams through the PE once, producing wh- and wl-partial
sums in PSUM partitions 0:64 / 64:128; all four split products accumulate.
The halves are summed token-major after the PE transpose. bf16 streams
1 cyc/row vs fp32's effective 4.

The hidden shard is pre-transposed on host ([H, tokens], contraction on
SBUF partitions) and laid out pass-major with 4 contraction chunks per DMA
so every hidden DMA is a single large fully-contiguous read.
"""

import sys
import numpy as np

sys.path.insert(0, "/opt/trn_rl_repo")

from contextlib import ExitStack

import concourse.bass as bass
import concourse.bacc as bacc
import concourse.mybir as mybir
import concourse.tile as tile
import concourse.masks as masks
from concourse.bass_utils import run_bass_kernel_spmd

F32 = mybir.dt.float32
BF16 = mybir.dt.bfloat16
U32 = mybir.dt.uint32
AF = mybir.ActivationFunctionType
ALU = mybir.AluOpType
AX = mybir.AxisListType

# Problem geometry (hardcoded per spec nn_AdaptiveRouter_50534585205486)
N, H, E = 16384, 4096, 64
NCORES = 8
NT = N // NCORES            # tokens per core (2048)
HC = H // 128               # contraction chunks (32)
QUAD = 4                    # contraction chunks per hidden DMA
EPS = 1e-8
TOPK = 2
# uneven passes: the last pass is small so the final (serial) epilogue is short
PASS_TOK = [1536, 512]
PASS_OFF = [0, 1536]


def build_nc():
    nc = bacc.Bacc("TRN2", target_bir_lowering=False, debug=False)

    # hidden hi/lo, one rectangular tensor per pass:
    # rows = (HC/QUAD groups) * 128 partitions, cols = QUAD * pass_tokens
    hts = []
    for p, tp in enumerate(PASS_TOK):
        hh = nc.dram_tensor(f"hth{p}", [HC // QUAD * 128, QUAD * tp], BF16,
                            kind="ExternalInput")
        hl = nc.dram_tensor(f"htl{p}", [HC // QUAD * 128, QUAD * tp], BF16,
                            kind="ExternalInput")
        hts.append((hh.rearrange("(c q) u -> c q u", q=128),
                    hl.rearrange("(c q) u -> c q u", q=128)))
    # packed per-chunk stationary: columns 0:E = wh, E:2E = wl
    wtp = nc.dram_tensor("wtp", [128, HC * 2 * E], BF16, kind="ExternalInput")
    imp = nc.dram_tensor("imp", [1, E], F32, kind="ExternalInput")
    temp = nc.dram_tensor("temp", [1, 1], F32, kind="ExternalInput")

    out0 = nc.dram_tensor("out0", [NT, E], F32, kind="ExternalOutput")
    # wi rows are ordered (partition, block) within each pass's token range
    out1 = nc.dram_tensor("out1", [NT, 4], F32, kind="ExternalOutput")
    pacc_d = nc.dram_tensor("pacc", [128, E], F32, kind="ExternalOutput")
    eacc_d = nc.dram_tensor("eacc", [128, 1], F32, kind="ExternalOutput")

    with ExitStack() as ctx:
        tc = ctx.enter_context(tile.TileContext(nc))
        cpool = ctx.enter_context(tc.tile_pool(name="const", bufs=1))
        hipool = ctx.enter_context(tc.tile_pool(name="hi", bufs=3))
        lopool = ctx.enter_context(tc.tile_pool(name="lo", bufs=3))
        acsbpool = ctx.enter_context(tc.tile_pool(name="acsb", bufs=2))
        tmpool = ctx.enter_context(tc.tile_pool(name="tm", bufs=2))
        spool = ctx.enter_context(tc.tile_pool(name="scratch", bufs=2))
        accpool = ctx.enter_context(tc.tile_pool(name="acc", bufs=1))
        ps_acc = ctx.enter_context(
            tc.tile_pool(name="psacc", bufs=1, space=bass.MemorySpace.PSUM)
        )
        ps_t = ctx.enter_context(
            tc.tile_pool(name="pst", bufs=2, space=bass.MemorySpace.PSUM)
        )

        # ---- weights first: the first matmul gates on these ------------
        wp = cpool.tile([128, HC, 2 * E], BF16)
        nc.scalar.dma_start(wp[:], wtp[:, :])

        timp = cpool.tile([1, E], F32)
        nc.sync.dma_start(timp[:], imp[:, :])
        ttemp = cpool.tile([1, 1], F32)
        nc.sync.dma_start(ttemp[:], temp[:, :])

        ident = cpool.tile([128, 128], F32)
        masks.make_identity(nc, ident[:])

        ones_row = cpool.tile([1, 128], F32)
        nc.vector.memset(ones_row[:], 1.0)
        eps1 = cpool.tile([1, 1], F32)
        nc.vector.memset(eps1[:], EPS)
        eps128 = cpool.tile([128, 1], F32)
        nc.vector.memset(eps128[:], EPS)
        zero128 = cpool.tile([128, 1], F32)
        nc.vector.memset(zero128[:], 0.0)

        # log(softmax(importance) + eps) on partition 0, scaled by 1/temp
        nm = cpool.tile([1, 1], F32)
        nc.vector.reduce_max(nm[:], timp[:], axis=AX.X, negate=True)
        te = cpool.tile([1, E], F32)
        nc.scalar.activation(te[:], timp[:], AF.Exp, bias=nm[:])
        tsum = cpool.tile([1, 1], F32)
        nc.vector.reduce_sum(tsum[:], te[:], axis=AX.X)
        trcp = cpool.tile([1, 1], F32)
        nc.vector.reciprocal(trcp[:], tsum[:])
        smx = cpool.tile([1, E], F32)
        nc.vector.tensor_scalar_mul(smx[:], te[:], trcp[:])
        lbrow = cpool.tile([1, E], F32)
        nc.scalar.activation(lbrow[:], smx[:], AF.Ln, bias=eps1[:])
        inv1 = cpool.tile([1, 1], F32)
        nc.vector.reciprocal(inv1[:], ttemp[:])
        lbs_row = cpool.tile([1, E], F32)
        nc.vector.tensor_scalar_mul(lbs_row[:], lbrow[:], inv1[:])

        # replicate bias*scale to all 128 partitions, and 1/temp to [128,1]
        lb_ps = ps_t.tile([128, E], F32, tag="pst")
        nc.tensor.matmul(lb_ps[:], ones_row[0:1, :], lbs_row[:], start=True, stop=True)
        lbs = cpool.tile([128, E], F32)
        nc.vector.tensor_copy(lbs[:], lb_ps[:])
        iv_ps = ps_t.tile([128, 1], F32, tag="pst")
        nc.tensor.matmul(iv_ps[:], ones_row[0:1, :], inv1[:], start=True, stop=True)
        s128 = cpool.tile([128, 1], F32)
        nc.vector.tensor_copy(s128[:], iv_ps[:])

        # global accumulators
        pacc = accpool.tile([128, E], F32)
        nc.vector.memset(pacc[:], 0.0)
        eacc = accpool.tile([128, 1], F32)
        nc.vector.memset(eacc[:], 0.0)

        # ---- main loop ------------------------------------------------
        for ps, tp in enumerate(PASS_TOK):
            blks = tp // 128
            hh_v, hl_v = hts[ps]
            acc = ps_acc.tile([128, tp], F32, tag=f"acc{ps}")
            for c4 in range(HC // QUAD):
                hi = hipool.tile([128, QUAD * tp], BF16, tag="hi")
                nc.sync.dma_start(hi[:], hh_v[c4])
                lo = lopool.tile([128, QUAD * tp], BF16, tag="lo")
                nc.scalar.dma_start(lo[:], hl_v[c4])
                for par in range(QUAD):
                    h = QUAD * c4 + par
                    for x_t, first, last in ((hi, True, False), (lo, False, True)):
                        for half in range(tp // 512):
                            nc.tensor.matmul(
                                acc[:, half * 512:(half + 1) * 512],
                                wp[:, h, :],
                                x_t[:, par * tp + half * 512:
                                    par * tp + (half + 1) * 512],
                                start=(h == 0 and first),
                                stop=(h == HC - 1 and last),
                            )

            # PSUM -> SBUF (ACT) so the PE can transpose it
            accsb = acsbpool.tile([128, tp], F32, tag="accsb")
            nc.scalar.copy(accsb[:], acc[:])

            # token-major: transpose 128x128 blocks; cols 0:E = wh part,
            # E:2E = wl part; fold + bias + scale into tm per block
            tm = tmpool.tile([128, blks, E], F32, tag="tm")
            for b in range(blks):
                tp_ps = ps_t.tile([128, 128], F32, tag="pst")
                nc.tensor.transpose(
                    tp_ps[:], accsb[:, b * 128:(b + 1) * 128], ident[:, :]
                )
                # fold halves + bias + scale via two chained STTs, each
                # reading one PSUM half: t = wl_part/temp + lb/temp, then
                # tm = wh_part/temp + t  (only one PSUM input per op)
                fold = spool.tile([128, E], F32, tag="fold")
                nc.vector.scalar_tensor_tensor(
                    fold[:], tp_ps[:, E:2 * E], s128[:], lbs[:],
                    op0=ALU.mult, op1=ALU.add,
                )
                nc.vector.scalar_tensor_tensor(
                    tm[:, b, 0:E], tp_ps[:, 0:E], s128[:], fold[:],
                    op0=ALU.mult, op1=ALU.add,
                )

            # logits stream out while the softmax/top-k chain runs
            o0 = PASS_OFF[ps]
            nc.sync.dma_start(
                out0[o0:o0 + tp, :].rearrange("(j q) c -> q j c", q=128), tm[:]
            )

            lg = tm[:, :, :]
            wi = tmpool.tile([128, blks, 4], F32, tag="wi")

            # softmax over experts
            nmax = spool.tile([128, blks], F32, tag="nmax")
            nc.vector.reduce_max(nmax[:], lg, axis=AX.X, negate=True)
            sh = spool.tile([128, blks, E], F32, tag="sh")
            nc.vector.tensor_tensor(
                sh[:], lg,
                nmax[:].rearrange("q (a o) -> q a o", o=1).broadcast_to((128, blks, E)),
                op=ALU.add,
            )
            ex = spool.tile([128, blks, E], F32, tag="ex")
            nc.scalar.activation(ex[:], sh[:], AF.Exp, bias=zero128[:])

            # top-2 per token (independent of the probs chain; its Exp is
            # issued next to the softmax Exp to avoid an ACT table swap)
            mx = spool.tile([128, blks, 8], F32, tag="mx")
            ix = spool.tile([128, blks, 8], U32, tag="ix")
            for b in range(blks):
                nc.vector.max(mx[:, b, :], tm[:, b, 0:E])
                nc.vector.max_index(ix[:, b, :], mx[:, b, :], tm[:, b, 0:E])
            d2 = spool.tile([128, blks, TOPK], F32, tag="d2")
            nc.vector.tensor_tensor(
                d2[:], mx[:, :, 0:TOPK],
                mx[:, :, 0:1].broadcast_to((128, blks, TOPK)),
                op=ALU.subtract,
            )
            e2 = spool.tile([128, blks, TOPK], F32, tag="e2")
            nc.scalar.activation(e2[:], d2[:], AF.Exp, bias=zero128[:])
            # tiny Ln on a slice of e2 preloads the ACT Ln table while the
            # DVE computes sums/recips — keeps the 1.3us table load off the
            # serial tail chain (reading e2 pins it after the Exp above)
            dummy_ln = spool.tile([1, 1], F32, tag="dummy")
            nc.scalar.activation(dummy_ln[:], e2[0:1, 0, 0:1], AF.Ln, bias=eps1[:])

            ssum = spool.tile([128, blks], F32, tag="ssum")
            nc.vector.reduce_sum(ssum[:], ex[:], axis=AX.X)
            rs = spool.tile([128, blks], F32, tag="rs")
            nc.vector.reciprocal(rs[:], ssum[:])
            pr = spool.tile([128, blks, E], F32, tag="pr")
            nc.vector.tensor_tensor(
                pr[:], ex[:],
                rs[:].rearrange("q (a o) -> q a o", o=1).broadcast_to((128, blks, E)),
                op=ALU.mult,
            )

            # entropy partial: sum over experts and blocks of p*log(p+eps)
            lp = spool.tile([128, blks, E], F32, tag="lp")
            nc.scalar.activation(lp[:], pr[:], AF.Ln, bias=eps128[:])
            pl = spool.tile([128, blks, E], F32, tag="pl")
            nc.vector.tensor_mul(pl[:], pr[:], lp[:])
            entp = spool.tile([128, 1], F32, tag="entp")
            nc.vector.reduce_sum(entp[:], pl[:], axis=AX.XY)
            nc.vector.tensor_add(eacc[:], eacc[:], entp[:])

            # expert-load partial: pairwise-tree sum of probs over blocks
            cur, w_ = pr, blks
            while w_ > 1:
                half = w_ // 2
                nxt = spool.tile([128, half, E], F32, tag=f"tree{half}_{ps}")
                nc.vector.tensor_add(nxt[:], cur[:, 0:half, :], cur[:, half:2 * half, :])
                if w_ % 2:
                    nc.vector.tensor_add(
                        nxt[:, 0:1, :], nxt[:, 0:1, :], cur[:, 2 * half:w_, :]
                    )
                cur, w_ = nxt, half
            nc.vector.tensor_add(pacc[:], pacc[:], cur[:, 0, :])

            s2 = spool.tile([128, blks], F32, tag="s2")
            nc.vector.reduce_sum(s2[:], e2[:], axis=AX.X)
            r2 = spool.tile([128, blks], F32, tag="r2")
            nc.vector.reciprocal(r2[:], s2[:])
            nc.vector.tensor_tensor(
                wi[:, :, 0:TOPK], e2[:],
                r2[:].rearrange("q (a o) -> q a o", o=1).broadcast_to((128, blks, TOPK)),
                op=ALU.mult,
            )
            # indices (uint32 -> f32 convert; values <= 63 are exact)
            nc.vector.tensor_copy(wi[:, :, TOPK:2 * TOPK], ix[:, :, 0:TOPK])

            nc.sync.dma_start(
                out1[o0:o0 + tp, :].rearrange("(q j) c -> q j c", j=blks), wi[:]
            )

        nc.sync.dma_start(pacc_d[:, :], pacc[:])
        nc.sync.dma_start(eacc_d[:, :], eacc[:])

    nc.compile()
    return nc


_NC_CACHE = None


def _get_nc():
    global _NC_CACHE
    if _NC_CACHE is None:
        _NC_CACHE = build_nc()
    return _NC_CACHE


def _split_bf16(x):
    import ml_dtypes
    hi = x.astype(ml_dtypes.bfloat16)
    lo = (x - hi.astype(np.float32)).astype(ml_dtypes.bfloat16)
    return hi, lo


def _quad_layout(block):
    """[H, tp] -> [(HC/QUAD)*128, QUAD*tp]: 4 contraction chunks per DMA row-block."""
    tp = block.shape[1]
    return np.ascontiguousarray(
        block.reshape(HC // QUAD, QUAD, 128, tp)
        .transpose(0, 2, 1, 3)
        .reshape(HC // QUAD * 128, QUAD * tp)
    )


def make_in_maps(hidden_states, router_weight, expert_importance, temperature):
    hs = np.ascontiguousarray(np.asarray(hidden_states, dtype=np.float32))
    # [E, H] -> [H, E] -> [HC, 128, E] -> [128, HC, E]
    wt = (
        np.asarray(router_weight, dtype=np.float32).T
        .reshape(HC, 128, E).transpose(1, 0, 2)
    )
    wth, wtl = _split_bf16(np.ascontiguousarray(wt))
    # pack [wh | wl] along the last axis -> [128, HC, 2E] -> [128, HC*2E]
    wtp = np.ascontiguousarray(
        np.concatenate([wth, wtl], axis=2).reshape(128, HC * 2 * E)
    )
    imp = np.asarray(expert_importance, dtype=np.float32).reshape(1, E)
    tmp = np.asarray(temperature, dtype=np.float32).reshape(1, 1)
    in_maps = []
    for c in range(NCORES):
        sh = hs[c * NT:(c + 1) * NT].T  # [H, NT]
        m = {"wtp": wtp, "imp": imp, "temp": tmp}
        for p, tp in enumerate(PASS_TOK):
            o = PASS_OFF[p]
            hi, lo = _split_bf16(np.ascontiguousarray(sh[:, o:o + tp]))
            m[f"hth{p}"] = _quad_layout(hi)
            m[f"htl{p}"] = _quad_layout(lo)
        in_maps.append(m)
    return in_maps


def postprocess(results):
    logits = np.empty((N, E), np.float32)
    idx = np.empty((N, TOPK), np.int32)
    ew = np.empty((N, TOPK), np.float32)
    load_sum = np.zeros(E, np.float64)
    ent_sum = 0.0
    for c, r in enumerate(results):
        logits[c * NT:(c + 1) * NT] = r["out0"]
        # out1 rows are (partition, block)-ordered within each pass range
        wi = np.empty((NT, 4), np.float32)
        for p, tp in enumerate(PASS_TOK):
            o = PASS_OFF[p]
            blks = tp // 128
            wi[o:o + tp] = (
                r["out1"][o:o + tp].reshape(128, blks, 4)
                .transpose(1, 0, 2).reshape(tp, 4)
            )
        ew[c * NT:(c + 1) * NT] = wi[:, 0:TOPK]
        idx[c * NT:(c + 1) * NT] = np.rint(wi[:, TOPK:2 * TOPK]).astype(np.int32)
        load_sum += r["pacc"].astype(np.float64).sum(axis=0)
        ent_sum += float(r["eacc"].astype(np.float64).sum())
    expert_load = (load_sum / N).astype(np.float32)
    load_var = np.float32(np.var(load_sum / N, ddof=1))
    entropy = np.float32(-ent_sum / N)
    return (logits, idx, ew, expert_load, load_var, entropy)


def kernel(hidden_states, router_weight, expert_importance, temperature, top_k):
    assert int(top_k) == TOPK
    nc = _get_nc()
    in_maps = make_in_maps(hidden_states, router_weight, expert_importance, temperature)
    res = run_bass_kernel_spmd(nc, in_maps, core_ids=list(range(NCORES)))
    return postprocess(res.results)
